# revision 32
# baseline (speedup 1.0000x reference)
"""Trainium2 Bass kernel for 2-layer LSTM (T=512, B=32, H=512), fp32 I/O.

Strategy: pure data-parallel over batch (8 cores x B_local=4, zero collectives).
Each core runs the full 2-layer scan for its batch slice.

Per-core design:
  - All on-chip layouts are "chunk-partitioned": SBUF/PSUM partition index
    p = 32*k + 4*r + b, where k = hidden-unit chunk (u div 128), r = replica
    (batch replicated 8x to fill the 128-wide PE stationary / all engine
    lanes), b = local batch. Free dim carries (gate, u_lo) for z-shaped
    tiles and u_lo for state-shaped tiles.
  - z matmuls: stationary = transposed hidden state (with replicas), moving =
    pre-transposed weight matrices; 4 PE column-groups stream the 4
    unit-chunks concurrently. Rounds are emitted round-robin across the
    column groups (group-major order would serialize the in-order PE queue).
    A K=5 "inject" round (emitted first: h-independent) adds x-projection
    + bias via a small identity-select stationary.
  - Gates: single ACT Tanh with scale=0.5 over [128, 512] using the
    tanh-half-trick (sigmoid(z) = 0.5*tanh(z/2)+0.5); the doubled-gate
    row scaling is folded into the weights on the host. Cell state is kept
    as S := 2c and hidden as H := 2h so the whole cell update is 4
    scalar_tensor_tensor ops; host folds the compensating 0.5 into all
    H-consuming weight columns and rescales the final output.
  - Each layer's PE transpose of h (the next step's stationary) is emitted
    at the top of that layer's NEXT step, so it never sits mid-queue
    waiting on the DVE gate tail while independent z rounds are blocked
    behind it. The z PSUM tiles are double-buffered so step t's inject can
    stream while step t-1's ACT still reads.
  - x-projections are big GEMMs: layer-1 fully precomputed to DRAM before
    the scan; layer-2 computed in 8-step blocks from transposed H1 tiles
    while the scan runs (layer-2 scan lags layer-1 by LAG=16 steps).
"""

import sys

if "/opt/trn_rl_repo" not in sys.path:
    sys.path.insert(0, "/opt/trn_rl_repo")

import numpy as np
import ml_dtypes

import concourse.bacc as bacc
import concourse.tile as tile
from concourse import mybir
from concourse.bass_utils import run_bass_kernel_spmd

T_STEPS = 512
B_LOC = 4
N_CORES = 8
H = 512
NG = 2048  # 4*H gate width
BLK = 8  # x2proj block size (steps)
LAG = 16  # layer-2 scan lag (steps)
BF = mybir.dt.bfloat16
F32 = mybir.dt.float32

_ADD = mybir.AluOpType.add
_MUL = mybir.AluOpType.mult
_TANH = mybir.ActivationFunctionType.Tanh


def _eff_stream(w, col_scale):
    """w: [2048, d] (gate-major rows i,f,g,o). Returns stream matrix
    [d, 2048] with rows scaled (g-gate x2 for the tanh trick), columns
    reordered chunk-major (n = 512*k + 128*g + u_lo), scaled by col_scale."""
    w2 = w.astype(np.float64) * col_scale
    w2[2 * H : 3 * H] *= 2.0
    wr = w2.reshape(4, 4, 128, w.shape[1])  # [g, k, u_lo, d]
    wr = wr.transpose(1, 0, 2, 3).reshape(NG, w.shape[1])
    return np.ascontiguousarray(wr.T.astype(np.float32))


def _eff_bias(b_ih, b_hh):
    b = (b_ih.astype(np.float64) + b_hh.astype(np.float64)).copy()
    b[2 * H : 3 * H] *= 2.0
    br = b.reshape(4, 4, 128).transpose(1, 0, 2).reshape(NG)
    return br.astype(np.float32)


def _bf16(x):
    return x.astype(ml_dtypes.bfloat16)


def _z_inject(nc, pz, is5, xp_sl):
    """The h-independent xproj inject round (emitted before the epilogue
    transpose so it streams while the PE would otherwise idle)."""
    for j in range(4):
        nc.tensor.matmul(
            pz[32 * j : 32 * j + 32, :],
            is5[:, :],
            xp_sl[0:4, 512 * j : 512 * j + 512],
            start=True,
            stop=False,
            tile_position=(0, 32 * j),
        )


def _z_krounds(nc, pz, ht, w):
    """The 4 recurrent accumulation rounds of z += W.T @ H, round-robin
    across the 4 column-tile groups so their moving streams overlap in
    the PE (group-major order serializes: each group's accumulation
    chain blocks the in-order queue head)."""
    for k in range(4):
        for j in range(4):
            nc.tensor.matmul(
                pz[32 * j : 32 * j + 32, :],
                ht[:, 32 * k : 32 * k + 32],
                w[:, NG * k + 512 * j : NG * k + 512 * j + 512],
                start=False,
                stop=(k == 3),
                tile_position=(0, 32 * j),
            )


def _gates(nc, pools, pz, s_prev, nm):
    """ACT+DVE cell update. Returns (s_new, h_new) tiles."""
    sbuf, state = pools
    t_sb = sbuf.tile([128, 512], BF, tag=f"t{nm}", name=f"t{nm}")
    nc.scalar.activation(t_sb[:], pz[:], _TANH, bias=0.0, scale=0.5)
    m2 = sbuf.tile([128, 128], F32, tag=f"m2{nm}", name=f"m2{nm}")
    nc.vector.scalar_tensor_tensor(
        m2[:], t_sb[:, 0:128], 1.0, t_sb[:, 256:384], op0=_ADD, op1=_MUL
    )
    m1 = sbuf.tile([128, 128], F32, tag=f"m1{nm}", name=f"m1{nm}")
    nc.vector.scalar_tensor_tensor(
        m1[:], t_sb[:, 128:256], 1.0, s_prev[:], op0=_ADD, op1=_MUL
    )
    s_new = state.tile([128, 128], F32, tag=f"s{nm}", name=f"s{nm}")
    nc.vector.scalar_tensor_tensor(
        s_new[:], m1[:], 0.5, m2[:], op0=_MUL, op1=_ADD
    )
    tc_sb = sbuf.tile([128, 128], BF, tag=f"tc{nm}", name=f"tc{nm}")
    nc.scalar.activation(tc_sb[:], s_new[:], _TANH, bias=0.0, scale=0.5)
    h_new = sbuf.tile([128, 128], BF, tag=f"h{nm}", name=f"h{nm}")
    nc.vector.scalar_tensor_tensor(
        h_new[:], t_sb[:, 384:512], 1.0, tc_sb[:], op0=_ADD, op1=_MUL
    )
    return s_new, h_new


def build_nc(t_steps=T_STEPS, repeat=1):
    nc = bacc.Bacc(
        "TRN2", target_bir_lowering=False, debug=False, num_devices=N_CORES
    )
    # kernel inputs (per-core)
    xt_d = nc.dram_tensor("xt", [H, t_steps * B_LOC], BF, kind="ExternalInput")
    w1i_d = nc.dram_tensor("w1i", [H, NG], BF, kind="ExternalInput")
    w1h_d = nc.dram_tensor("w1h", [H, NG], BF, kind="ExternalInput")
    w2i_d = nc.dram_tensor("w2i", [H, NG], BF, kind="ExternalInput")
    w2h_d = nc.dram_tensor("w2h", [H, NG], BF, kind="ExternalInput")
    b1_d = nc.dram_tensor("b1r", [1, NG], BF, kind="ExternalInput")
    b2_d = nc.dram_tensor("b2r", [1, NG], BF, kind="ExternalInput")
    is5_d = nc.dram_tensor("is4", [4, 32], BF, kind="ExternalInput")
    idb_d = nc.dram_tensor("idb", [128, 128], BF, kind="ExternalInput")
    y_d = nc.dram_tensor("y", [t_steps, 2048], BF, kind="ExternalOutput")

    rb_sz = min(128, t_steps * B_LOC)  # phase-1 row-block size
    n_tb = t_steps * B_LOC // rb_sz

    with tile.TileContext(nc) as tc:
        with (
            tc.tile_pool(name="const", bufs=1) as const,
            tc.tile_pool(name="state", bufs=2) as state,
            tc.tile_pool(name="work", bufs=2) as work,
            tc.tile_pool(name="xp1p", bufs=3) as xp1p,
            tc.tile_pool(name="xp2p", bufs=3) as xp2p,
            tc.tile_pool(name="hblkp", bufs=2) as hblkp,
            tc.tile_pool(name="pzp", bufs=2, space="PSUM") as pzp,
            tc.tile_pool(name="ptp", bufs=1, space="PSUM") as ptp,
            tc.tile_pool(name="jnkp", bufs=1, space="PSUM") as jnkp,
            tc.tile_pool(name="dram", bufs=1, space="DRAM") as dramp,
        ):
            # ---- constants / weights to SBUF
            is5 = const.tile([4, 32], BF, name="is4")
            nc.sync.dma_start(is5[:], is5_d.ap())
            b1sb = const.tile([1, NG], BF, name="b1sb")
            nc.sync.dma_start(b1sb[:], b1_d.ap())
            b2sb = const.tile([1, NG], BF, name="b2sb")
            nc.sync.dma_start(b2sb[:], b2_d.ap())
            ones1 = const.tile([1, 128], BF, name="ones1")
            nc.gpsimd.memset(ones1[:], 1.0)
            idb = const.tile([128, 128], BF)
            nc.sync.dma_start(idb[:], idb_d.ap())

            w1h = const.tile([128, 4 * NG], BF, name="w1h")
            w2i = const.tile([128, 4 * NG], BF, name="w2i")
            w2h = const.tile([128, 4 * NG], BF, name="w2h")
            for w_sb, w_dd in ((w1h, w1h_d), (w2i, w2i_d), (w2h, w2h_d)):
                for k in range(4):
                    nc.sync.dma_start(
                        w_sb[:, NG * k : NG * k + NG],
                        w_dd[128 * k : 128 * k + 128, :],
                    )

            # DRAM scratch
            x1d = dramp.tile([t_steps * B_LOC, NG], BF, name="x1d")
            x2d = dramp.tile([t_steps * B_LOC, NG], BF, name="x2d")

            for _rep in range(repeat):
              # ---- phase 1: x1proj GEMM -> DRAM
              with tc.tile_pool(name="ph1", bufs=2) as ph1:
                  w1i = ph1.tile([128, 4 * NG], BF, bufs=1, name="w1i")
                  xts = ph1.tile([128, 4 * t_steps * B_LOC], BF, bufs=1,
                                 name="xts")
                  for k in range(4):
                      nc.sync.dma_start(
                          w1i[:, NG * k : NG * k + NG],
                          w1i_d[128 * k : 128 * k + 128, :],
                      )
                      nc.sync.dma_start(
                          xts[:, t_steps * B_LOC * k : t_steps * B_LOC * (k + 1)],
                          xt_d[128 * k : 128 * k + 128, :],
                      )
                  for tb in range(n_tb):
                      cpx = ph1.tile([rb_sz, NG], BF, name="cpx")
                      for nj in range(4):
                          # borrow the scan's double-buffered z PSUM tags
                          pxa = pzp.tile([rb_sz, 512], F32,
                                         tag=("pz1" if nj % 2 == 0 else "pz2"),
                                         name="pxa")
                          nc.tensor.matmul(
                              pxa[:],
                              ones1[0:1, 0:rb_sz],
                              b1sb[0:1, 512 * nj : 512 * nj + 512],
                              start=True,
                              stop=False,
                          )
                          for k in range(4):
                              nc.tensor.matmul(
                                  pxa[:],
                                  xts[:, t_steps * B_LOC * k + rb_sz * tb :
                                      t_steps * B_LOC * k + rb_sz * tb + rb_sz],
                                  w1i[:, NG * k + 512 * nj : NG * k + 512 * nj + 512],
                                  start=False,
                                  stop=(k == 3),
                              )
                          nc.vector.tensor_copy(
                              cpx[:, 512 * nj : 512 * nj + 512], pxa[:]
                          )
                      nc.sync.dma_start(
                          x1d[rb_sz * tb : rb_sz * tb + rb_sz, :], cpx[:]
                      )

              # ---- initial states
              s1 = state.tile([128, 128], F32, tag="s1", name="s1")
              nc.gpsimd.memset(s1[:], 0.0)
              s2 = state.tile([128, 128], F32, tag="s2", name="s2")
              nc.gpsimd.memset(s2[:], 0.0)
              ht1 = state.tile([128, 128], BF, tag="ht1", name="ht1")
              nc.gpsimd.memset(ht1[:], 0.0)
              ht2 = state.tile([128, 128], BF, tag="ht2", name="ht2")
              nc.gpsimd.memset(ht2[:], 0.0)

              h1c = h2c = hblk = None
              x2_pending = None
              pools = (work, state)

              # HAM warm-keeper: the PE idles ~0.8us before each epilogue
              # transpose (waiting on the DVE gate tail), which re-throttles
              # the PE clock to 1.2GHz. Full-array junk matmuls (K=M=128 so
              # the whole array toggles — small-K/M ops don't register with
              # the activity monitor) in those gaps keep the clock at 2.4GHz;
              # their results are never read.
              junk = jnkp.tile([128, 512], F32, name="junk")

              def _warm(n):
                  for _ in range(n):
                      nc.tensor.matmul(
                          junk[:],
                          idb[:],
                          w1h[:, 0:512],
                          start=True,
                          stop=True,
                      )

              # ---- fused scan (layer epilogues rephased to the next step)
              for tau in range(t_steps + LAG + 1):
                  # --- L1 epilogue for step tau-1: transpose h -> new
                  # stationary, append to the x2proj h-block, fire the
                  # block GEMM when the block completes.
                  t1p = tau - 1
                  if 0 <= t1p < t_steps:
                      pt1 = ptp.tile([128, 128], BF, tag="pt1", name="pt1")
                      nc.tensor.transpose(pt1[:], h1c[:], idb[:])
                      ht1 = state.tile([128, 128], BF, tag="ht1", name="ht1")
                      nc.vector.tensor_copy(ht1[:], pt1[:])
                      bi, off = divmod(t1p, BLK)
                      nc.vector.tensor_copy(
                          hblk[:].rearrange("p (k t b) -> p k t b", k=4, b=4)
                          [:, :, off, :],
                          ht1[:].rearrange("p (k rb) -> p k rb", rb=32)
                          [:, :, 0:4],
                      )
                      if off == BLK - 1:
                          x2_pending = (bi, hblk)

                  # --- L1 step tau
                  if tau < t_steps:
                      t1i = tau
                      if t1i % BLK == 0:
                          hblk = hblkp.tile([128, BLK * 16], BF, tag="hblk",
                                            name="hblk")
                      xp1 = xp1p.tile([B_LOC, NG], BF, tag="xp1", name="xp1")
                      nc.sync.dma_start(
                          xp1[:], x1d[B_LOC * t1i : B_LOC * t1i + B_LOC, :]
                      )
                      pz1 = pzp.tile([128, 512], F32, tag="pz1", name="pz1")
                      _z_inject(nc, pz1, is5, xp1)
                      _z_krounds(nc, pz1, ht1, w1h)
                      s1, h1c = _gates(nc, pools, pz1, s1, "L1")
                      _warm(3)

                  # --- L2 epilogue for step tau-LAG-1: transpose h, emit y
                  t2p = tau - LAG - 1
                  if 0 <= t2p < t_steps:
                      pt2 = ptp.tile([128, 128], BF, tag="pt2", name="pt2")
                      nc.tensor.transpose(pt2[:], h2c[:], idb[:])
                      ht2 = state.tile([128, 128], BF, tag="ht2", name="ht2")
                      nc.vector.tensor_copy(ht2[:], pt2[:])
                      yst = work.tile([128, 16], BF, tag="yst", name="yst")
                      nc.vector.tensor_copy(
                          yst[:].rearrange("u (k b) -> u k b", b=4),
                          ht2[:].rearrange("u (k rb) -> u k rb", rb=32)
                          [:, :, 0:4],
                      )
                      nc.sync.dma_start(
                          y_d[t2p : t2p + 1, :]
                          .rearrange("o (u f) -> (o u) f", u=128),
                          yst[:],
                      )

                  # --- L2 step tau-LAG
                  t2i = tau - LAG
                  if 0 <= t2i < t_steps:
                      xp2 = xp2p.tile([B_LOC, NG], BF, tag="xp2", name="xp2")
                      nc.sync.dma_start(
                          xp2[:], x2d[B_LOC * t2i : B_LOC * t2i + B_LOC, :]
                      )
                      pz2 = pzp.tile([128, 512], F32, tag="pz2", name="pz2")
                      _z_inject(nc, pz2, is5, xp2)
                      _z_krounds(nc, pz2, ht2, w2h)
                      s2, h2c = _gates(nc, pools, pz2, s2, "L2")
                      if x2_pending is None:
                          _warm(3)

                  # --- x2proj GEMM for a completed h-block (emitted last so
                  # it never delays the chain-critical z matmuls; it is
                  # consumed LAG-BLK steps later). 4 col-tiled groups,
                  # round-robin (output chunk nj at PSUM partitions 32*nj).
                  if x2_pending is not None:
                      bi, hb = x2_pending
                      x2_pending = None
                      hbr = hb[:].rearrange("p (k tb) -> p k tb", k=4)
                      cx2 = work.tile([32, NG], BF, tag="cx2", name="cx2")
                      pxb = pzp.tile([128, 512], F32, tag="pxb", bufs=1,
                                     name="pxb")
                      for nj in range(4):
                          nc.tensor.matmul(
                              pxb[32 * nj : 32 * nj + 32, :],
                              ones1[0:1, 0:32],
                              b2sb[0:1, 512 * nj : 512 * nj + 512],
                              start=True,
                              stop=False,
                              tile_position=(0, 32 * nj),
                          )
                      for k in range(4):
                          for nj in range(4):
                              nc.tensor.matmul(
                                  pxb[32 * nj : 32 * nj + 32, :],
                                  hbr[:, k, :],
                                  w2i[:, NG * k + 512 * nj :
                                      NG * k + 512 * nj + 512],
                                  start=False,
                                  stop=(k == 3),
                                  tile_position=(0, 32 * nj),
                              )
                      for nj in range(4):
                          nc.vector.tensor_copy(
                              cx2[:, 512 * nj : 512 * nj + 512],
                              pxb[32 * nj : 32 * nj + 32, :],
                          )
                      nc.sync.dma_start(
                          x2d[B_LOC * BLK * bi : B_LOC * BLK * (bi + 1), :],
                          cx2[:],
                      )
    nc.compile()
    return nc


def host_inputs(seq_inputs, W_ih, W_hh, b_ih, b_hh, t_steps=T_STEPS):
    """Build the 8 per-core input maps."""
    w1i = _bf16(_eff_stream(W_ih[0], 1.0))
    w1h = _bf16(_eff_stream(W_hh[0], 0.5))
    w2i = _bf16(_eff_stream(W_ih[1], 0.5))
    w2h = _bf16(_eff_stream(W_hh[1], 0.5))
    b1 = _bf16(_eff_bias(b_ih[0], b_hh[0]))[None, :]
    b2 = _bf16(_eff_bias(b_ih[1], b_hh[1]))[None, :]
    is4 = np.zeros((4, 32), np.float32)
    for r in range(8):
        for b in range(B_LOC):
            is4[b, 4 * r + b] = 1.0
    is4 = _bf16(is4)
    idb = _bf16(np.eye(128, dtype=np.float32))

    in_maps = []
    for c in range(N_CORES):
        xs = seq_inputs[:t_steps, B_LOC * c : B_LOC * (c + 1), :]  # [T,4,512]
        xt = np.ascontiguousarray(
            xs.transpose(2, 0, 1).reshape(H, t_steps * B_LOC)
        )
        in_maps.append(
            {
                "xt": _bf16(xt),
                "w1i": w1i,
                "w1h": w1h,
                "w2i": w2i,
                "w2h": w2h,
                "b1r": b1,
                "b2r": b2,
                "is4": is4,
                "idb": idb,
            }
        )
    return in_maps


def gather_output(results, t_steps=T_STEPS):
    B = B_LOC * N_CORES
    y = np.empty((t_steps, B, H), np.float32)
    for c in range(N_CORES):
        yc = np.asarray(results[c]["y"], dtype=np.float32)
        yc = yc.reshape(t_steps, 128, 4, 4)  # [t, u_lo, k, b]
        yc = yc.transpose(0, 3, 2, 1).reshape(t_steps, B_LOC, H)
        y[:, B_LOC * c : B_LOC * (c + 1), :] = yc
    return 0.5 * y  # H2 = 2*h2


_NC_CACHE = {}


def kernel(seq_inputs, W_ih, W_hh, b_ih, b_hh):
    seq_inputs = np.asarray(seq_inputs, np.float32)
    W_ih = np.asarray(W_ih, np.float32)
    W_hh = np.asarray(W_hh, np.float32)
    b_ih = np.asarray(b_ih, np.float32)
    b_hh = np.asarray(b_hh, np.float32)
    t_steps = seq_inputs.shape[0]
    if t_steps not in _NC_CACHE:
        _NC_CACHE[t_steps] = build_nc(t_steps)
    nc = _NC_CACHE[t_steps]
    in_maps = host_inputs(seq_inputs, W_ih, W_hh, b_ih, b_hh, t_steps)
    res = run_bass_kernel_spmd(nc, in_maps, core_ids=list(range(N_CORES)))
    return gather_output(res.results, t_steps)


# revision 33
# speedup vs baseline: 1.1425x; 1.1425x over previous
"""Trainium2 Bass kernel for 2-layer LSTM (T=512, B=32, H=512), fp32 I/O.

Strategy: pure data-parallel over batch (8 cores x B_local=4, zero collectives).
Each core runs the full 2-layer scan for its batch slice.

Per-core design:
  - All on-chip layouts are "chunk-partitioned": SBUF/PSUM partition index
    p = 32*k + 4*r + b, where k = hidden-unit chunk (u div 128), r = replica
    (batch replicated 8x to fill the 128-wide PE stationary / all engine
    lanes), b = local batch. Free dim carries (gate, u_lo) for z-shaped
    tiles and u_lo for state-shaped tiles.
  - z matmuls: stationary = transposed hidden state (with replicas), moving =
    pre-transposed weight matrices; 4 PE column-groups stream the 4
    unit-chunks concurrently. Rounds are emitted round-robin across the
    column groups (group-major order would serialize the in-order PE queue).
    A K=5 "inject" round (emitted first: h-independent) adds x-projection
    + bias via a small identity-select stationary.
  - Gates: single ACT Tanh with scale=0.5 over [128, 512] using the
    tanh-half-trick (sigmoid(z) = 0.5*tanh(z/2)+0.5); the doubled-gate
    row scaling is folded into the weights on the host. Cell state is kept
    as S := 2c and hidden as H := 2h so the whole cell update is 4
    scalar_tensor_tensor ops; host folds the compensating 0.5 into all
    H-consuming weight columns and rescales the final output.
  - Each layer's PE transpose of h (the next step's stationary) is emitted
    at the top of that layer's NEXT step, so it never sits mid-queue
    waiting on the DVE gate tail while independent z rounds are blocked
    behind it. The z PSUM tiles are double-buffered so step t's inject can
    stream while step t-1's ACT still reads.
  - x-projections are big GEMMs: layer-1 fully precomputed to DRAM before
    the scan; layer-2 computed in 8-step blocks from transposed H1 tiles
    while the scan runs (layer-2 scan lags layer-1 by LAG=16 steps).
"""

import sys

if "/opt/trn_rl_repo" not in sys.path:
    sys.path.insert(0, "/opt/trn_rl_repo")

import numpy as np
import ml_dtypes

import concourse.bacc as bacc
import concourse.tile as tile
from concourse import mybir
from concourse.bass_utils import run_bass_kernel_spmd

T_STEPS = 512
B_LOC = 4
N_CORES = 8
H = 512
NG = 2048  # 4*H gate width
BLK = 8  # x2proj block size (steps)
LAG = 16  # layer-2 scan lag (steps)
BF = mybir.dt.bfloat16
F32 = mybir.dt.float32

_ADD = mybir.AluOpType.add
_MUL = mybir.AluOpType.mult
_TANH = mybir.ActivationFunctionType.Tanh


def _eff_stream(w, col_scale):
    """w: [2048, d] (gate-major rows i,f,g,o). Returns stream matrix
    [d, 2048] with rows scaled (g-gate x2 for the tanh trick), columns
    reordered chunk-major (n = 512*k + 128*g + u_lo), scaled by col_scale."""
    w2 = w.astype(np.float64) * col_scale
    w2[2 * H : 3 * H] *= 2.0
    wr = w2.reshape(4, 4, 128, w.shape[1])  # [g, k, u_lo, d]
    wr = wr.transpose(1, 0, 2, 3).reshape(NG, w.shape[1])
    return np.ascontiguousarray(wr.T.astype(np.float32))


def _eff_bias(b_ih, b_hh):
    b = (b_ih.astype(np.float64) + b_hh.astype(np.float64)).copy()
    b[2 * H : 3 * H] *= 2.0
    br = b.reshape(4, 4, 128).transpose(1, 0, 2).reshape(NG)
    return br.astype(np.float32)


def _bf16(x):
    return x.astype(ml_dtypes.bfloat16)


def _z_inject(nc, pz, is5, xp_sl):
    """The h-independent xproj inject round (emitted before the epilogue
    transpose so it streams while the PE would otherwise idle)."""
    for j in range(4):
        nc.tensor.matmul(
            pz[32 * j : 32 * j + 32, :],
            is5[:, :],
            xp_sl[0:4, 512 * j : 512 * j + 512],
            start=True,
            stop=False,
            tile_position=(0, 32 * j),
        )


def _z_krounds(nc, pz, ht, w):
    """The 4 recurrent accumulation rounds of z += W.T @ H, round-robin
    across the 4 column-tile groups so their moving streams overlap in
    the PE (group-major order serializes: each group's accumulation
    chain blocks the in-order queue head)."""
    for k in range(4):
        for j in range(4):
            nc.tensor.matmul(
                pz[32 * j : 32 * j + 32, :],
                ht[:, 32 * k : 32 * k + 32],
                w[:, NG * k + 512 * j : NG * k + 512 * j + 512],
                start=False,
                stop=(k == 3),
                tile_position=(0, 32 * j),
            )


def _gates(nc, pools, pz, s_prev, nm):
    """ACT+DVE cell update. Returns (s_new, h_new) tiles."""
    sbuf, state = pools
    t_sb = sbuf.tile([128, 512], BF, tag=f"t{nm}", name=f"t{nm}")
    nc.scalar.activation(t_sb[:], pz[:], _TANH, bias=0.0, scale=0.5)
    m2 = sbuf.tile([128, 128], F32, tag=f"m2{nm}", name=f"m2{nm}")
    nc.vector.scalar_tensor_tensor(
        m2[:], t_sb[:, 0:128], 1.0, t_sb[:, 256:384], op0=_ADD, op1=_MUL
    )
    m1 = sbuf.tile([128, 128], F32, tag=f"m1{nm}", name=f"m1{nm}")
    nc.vector.scalar_tensor_tensor(
        m1[:], t_sb[:, 128:256], 1.0, s_prev[:], op0=_ADD, op1=_MUL
    )
    s_new = state.tile([128, 128], F32, tag=f"s{nm}", name=f"s{nm}")
    nc.vector.scalar_tensor_tensor(
        s_new[:], m1[:], 0.5, m2[:], op0=_MUL, op1=_ADD
    )
    tc_sb = sbuf.tile([128, 128], BF, tag=f"tc{nm}", name=f"tc{nm}")
    nc.scalar.activation(tc_sb[:], s_new[:], _TANH, bias=0.0, scale=0.5)
    h_new = sbuf.tile([128, 128], BF, tag=f"h{nm}", name=f"h{nm}")
    nc.vector.scalar_tensor_tensor(
        h_new[:], t_sb[:, 384:512], 1.0, tc_sb[:], op0=_ADD, op1=_MUL
    )
    return s_new, h_new


def build_nc(t_steps=T_STEPS, repeat=1):
    nc = bacc.Bacc(
        "TRN2", target_bir_lowering=False, debug=False, num_devices=N_CORES
    )
    # kernel inputs (per-core)
    xt_d = nc.dram_tensor("xt", [H, t_steps * B_LOC], BF, kind="ExternalInput")
    w1i_d = nc.dram_tensor("w1i", [H, NG], BF, kind="ExternalInput")
    w1h_d = nc.dram_tensor("w1h", [H, NG], BF, kind="ExternalInput")
    w2i_d = nc.dram_tensor("w2i", [H, NG], BF, kind="ExternalInput")
    w2h_d = nc.dram_tensor("w2h", [H, NG], BF, kind="ExternalInput")
    b1_d = nc.dram_tensor("b1r", [1, NG], BF, kind="ExternalInput")
    b2_d = nc.dram_tensor("b2r", [1, NG], BF, kind="ExternalInput")
    is5_d = nc.dram_tensor("is4", [4, 32], BF, kind="ExternalInput")
    idb_d = nc.dram_tensor("idb", [128, 128], BF, kind="ExternalInput")
    y_d = nc.dram_tensor("y", [t_steps, 2048], BF, kind="ExternalOutput")

    rb_sz = min(128, t_steps * B_LOC)  # phase-1 row-block size
    n_tb = t_steps * B_LOC // rb_sz

    with tile.TileContext(nc) as tc:
        with (
            tc.tile_pool(name="const", bufs=1) as const,
            tc.tile_pool(name="state", bufs=2) as state,
            tc.tile_pool(name="work", bufs=2) as work,
            tc.tile_pool(name="xp1p", bufs=3) as xp1p,
            tc.tile_pool(name="xp2p", bufs=3) as xp2p,
            tc.tile_pool(name="hblkp", bufs=2) as hblkp,
            tc.tile_pool(name="pzp", bufs=2, space="PSUM") as pzp,
            tc.tile_pool(name="ptp", bufs=1, space="PSUM") as ptp,
            tc.tile_pool(name="jnkp", bufs=1, space="PSUM") as jnkp,
            tc.tile_pool(name="dram", bufs=1, space="DRAM") as dramp,
        ):
            # ---- constants / weights to SBUF
            is5 = const.tile([4, 32], BF, name="is4")
            nc.sync.dma_start(is5[:], is5_d.ap())
            b1sb = const.tile([1, NG], BF, name="b1sb")
            nc.sync.dma_start(b1sb[:], b1_d.ap())
            b2sb = const.tile([1, NG], BF, name="b2sb")
            nc.sync.dma_start(b2sb[:], b2_d.ap())
            ones1 = const.tile([1, 128], BF, name="ones1")
            nc.gpsimd.memset(ones1[:], 1.0)
            idb = const.tile([128, 128], BF)
            nc.sync.dma_start(idb[:], idb_d.ap())

            w1h = const.tile([128, 4 * NG], BF, name="w1h")
            w2i = const.tile([128, 4 * NG], BF, name="w2i")
            w2h = const.tile([128, 4 * NG], BF, name="w2h")
            for w_sb, w_dd in ((w1h, w1h_d), (w2i, w2i_d), (w2h, w2h_d)):
                for k in range(4):
                    nc.sync.dma_start(
                        w_sb[:, NG * k : NG * k + NG],
                        w_dd[128 * k : 128 * k + 128, :],
                    )

            # DRAM scratch
            x1d = dramp.tile([t_steps * B_LOC, NG], BF, name="x1d")
            x2d = dramp.tile([t_steps * B_LOC, NG], BF, name="x2d")

            for _rep in range(repeat):
              # ---- phase 1: x1proj GEMM -> DRAM
              with tc.tile_pool(name="ph1", bufs=2) as ph1:
                  w1i = ph1.tile([128, 4 * NG], BF, bufs=1, name="w1i")
                  xts = ph1.tile([128, 4 * t_steps * B_LOC], BF, bufs=1,
                                 name="xts")
                  for k in range(4):
                      nc.sync.dma_start(
                          w1i[:, NG * k : NG * k + NG],
                          w1i_d[128 * k : 128 * k + 128, :],
                      )
                      nc.sync.dma_start(
                          xts[:, t_steps * B_LOC * k : t_steps * B_LOC * (k + 1)],
                          xt_d[128 * k : 128 * k + 128, :],
                      )
                  for tb in range(n_tb):
                      cpx = ph1.tile([rb_sz, NG], BF, name="cpx")
                      for nj in range(4):
                          # borrow the scan's double-buffered z PSUM tags
                          pxa = pzp.tile([rb_sz, 512], F32,
                                         tag=("pz1" if nj % 2 == 0 else "pz2"),
                                         name="pxa")
                          nc.tensor.matmul(
                              pxa[:],
                              ones1[0:1, 0:rb_sz],
                              b1sb[0:1, 512 * nj : 512 * nj + 512],
                              start=True,
                              stop=False,
                          )
                          for k in range(4):
                              nc.tensor.matmul(
                                  pxa[:],
                                  xts[:, t_steps * B_LOC * k + rb_sz * tb :
                                      t_steps * B_LOC * k + rb_sz * tb + rb_sz],
                                  w1i[:, NG * k + 512 * nj : NG * k + 512 * nj + 512],
                                  start=False,
                                  stop=(k == 3),
                              )
                          nc.vector.tensor_copy(
                              cpx[:, 512 * nj : 512 * nj + 512], pxa[:]
                          )
                      nc.sync.dma_start(
                          x1d[rb_sz * tb : rb_sz * tb + rb_sz, :], cpx[:]
                      )

              # ---- initial states
              s1 = state.tile([128, 128], F32, tag="s1", name="s1")
              nc.gpsimd.memset(s1[:], 0.0)
              s2 = state.tile([128, 128], F32, tag="s2", name="s2")
              nc.gpsimd.memset(s2[:], 0.0)
              ht1 = state.tile([128, 128], BF, tag="ht1", name="ht1")
              nc.gpsimd.memset(ht1[:], 0.0)
              ht2 = state.tile([128, 128], BF, tag="ht2", name="ht2")
              nc.gpsimd.memset(ht2[:], 0.0)

              h1c = h2c = hblk = None
              x2_pending = None
              pools = (work, state)

              # HAM warm-keeper: the PE idles ~0.8us before each epilogue
              # transpose (waiting on the DVE gate tail), which re-throttles
              # the PE clock to 1.2GHz. Full-array junk matmuls (K=M=128 so
              # the whole array toggles — small-K/M ops don't register with
              # the activity monitor) in those gaps keep the clock at 2.4GHz;
              # their results are never read.
              junk = jnkp.tile([128, 384], F32, name="junk")

              def _warm(n):
                  for _ in range(n):
                      nc.tensor.matmul(
                          junk[:],
                          idb[:],
                          w1h[:, 0:384],
                          start=True,
                          stop=True,
                      )

              # ---- fused scan (layer epilogues rephased to the next step)
              for tau in range(t_steps + LAG + 1):
                  # --- L1 epilogue for step tau-1: transpose h -> new
                  # stationary, append to the x2proj h-block, fire the
                  # block GEMM when the block completes.
                  t1p = tau - 1
                  if 0 <= t1p < t_steps:
                      pt1 = ptp.tile([128, 128], BF, tag="pt1", name="pt1")
                      nc.tensor.transpose(pt1[:], h1c[:], idb[:])
                      ht1 = state.tile([128, 128], BF, tag="ht1", name="ht1")
                      nc.vector.tensor_copy(ht1[:], pt1[:])
                      bi, off = divmod(t1p, BLK)
                      nc.vector.tensor_copy(
                          hblk[:].rearrange("p (k t b) -> p k t b", k=4, b=4)
                          [:, :, off, :],
                          ht1[:].rearrange("p (k rb) -> p k rb", rb=32)
                          [:, :, 0:4],
                      )
                      if off == BLK - 1:
                          x2_pending = (bi, hblk)

                  # --- L1 step tau
                  if tau < t_steps:
                      t1i = tau
                      if t1i % BLK == 0:
                          hblk = hblkp.tile([128, BLK * 16], BF, tag="hblk",
                                            name="hblk")
                      xp1 = xp1p.tile([B_LOC, NG], BF, tag="xp1", name="xp1")
                      nc.sync.dma_start(
                          xp1[:], x1d[B_LOC * t1i : B_LOC * t1i + B_LOC, :]
                      )
                      pz1 = pzp.tile([128, 512], F32, tag="pz1", name="pz1")
                      _z_inject(nc, pz1, is5, xp1)
                      _z_krounds(nc, pz1, ht1, w1h)
                      s1, h1c = _gates(nc, pools, pz1, s1, "L1")
                      _warm(2)

                  # --- L2 epilogue for step tau-LAG-1: transpose h, emit y
                  t2p = tau - LAG - 1
                  if 0 <= t2p < t_steps:
                      pt2 = ptp.tile([128, 128], BF, tag="pt2", name="pt2")
                      nc.tensor.transpose(pt2[:], h2c[:], idb[:])
                      ht2 = state.tile([128, 128], BF, tag="ht2", name="ht2")
                      nc.vector.tensor_copy(ht2[:], pt2[:])
                      yst = work.tile([128, 16], BF, tag="yst", name="yst")
                      nc.vector.tensor_copy(
                          yst[:].rearrange("u (k b) -> u k b", b=4),
                          ht2[:].rearrange("u (k rb) -> u k rb", rb=32)
                          [:, :, 0:4],
                      )
                      nc.sync.dma_start(
                          y_d[t2p : t2p + 1, :]
                          .rearrange("o (u f) -> (o u) f", u=128),
                          yst[:],
                      )

                  # --- L2 step tau-LAG
                  t2i = tau - LAG
                  if 0 <= t2i < t_steps:
                      xp2 = xp2p.tile([B_LOC, NG], BF, tag="xp2", name="xp2")
                      nc.sync.dma_start(
                          xp2[:], x2d[B_LOC * t2i : B_LOC * t2i + B_LOC, :]
                      )
                      pz2 = pzp.tile([128, 512], F32, tag="pz2", name="pz2")
                      _z_inject(nc, pz2, is5, xp2)
                      _z_krounds(nc, pz2, ht2, w2h)
                      s2, h2c = _gates(nc, pools, pz2, s2, "L2")
                      if x2_pending is None:
                          _warm(2)

                  # --- x2proj GEMM for a completed h-block (emitted last so
                  # it never delays the chain-critical z matmuls; it is
                  # consumed LAG-BLK steps later). 4 col-tiled groups,
                  # round-robin (output chunk nj at PSUM partitions 32*nj).
                  if x2_pending is not None:
                      bi, hb = x2_pending
                      x2_pending = None
                      hbr = hb[:].rearrange("p (k tb) -> p k tb", k=4)
                      cx2 = work.tile([32, NG], BF, tag="cx2", name="cx2")
                      pxb = pzp.tile([128, 512], F32, tag="pxb", bufs=1,
                                     name="pxb")
                      for nj in range(4):
                          nc.tensor.matmul(
                              pxb[32 * nj : 32 * nj + 32, :],
                              ones1[0:1, 0:32],
                              b2sb[0:1, 512 * nj : 512 * nj + 512],
                              start=True,
                              stop=False,
                              tile_position=(0, 32 * nj),
                          )
                      for k in range(4):
                          for nj in range(4):
                              nc.tensor.matmul(
                                  pxb[32 * nj : 32 * nj + 32, :],
                                  hbr[:, k, :],
                                  w2i[:, NG * k + 512 * nj :
                                      NG * k + 512 * nj + 512],
                                  start=False,
                                  stop=(k == 3),
                                  tile_position=(0, 32 * nj),
                              )
                      for nj in range(4):
                          nc.vector.tensor_copy(
                              cx2[:, 512 * nj : 512 * nj + 512],
                              pxb[32 * nj : 32 * nj + 32, :],
                          )
                      nc.sync.dma_start(
                          x2d[B_LOC * BLK * bi : B_LOC * BLK * (bi + 1), :],
                          cx2[:],
                      )
    nc.compile()
    return nc


def host_inputs(seq_inputs, W_ih, W_hh, b_ih, b_hh, t_steps=T_STEPS):
    """Build the 8 per-core input maps."""
    w1i = _bf16(_eff_stream(W_ih[0], 1.0))
    w1h = _bf16(_eff_stream(W_hh[0], 0.5))
    w2i = _bf16(_eff_stream(W_ih[1], 0.5))
    w2h = _bf16(_eff_stream(W_hh[1], 0.5))
    b1 = _bf16(_eff_bias(b_ih[0], b_hh[0]))[None, :]
    b2 = _bf16(_eff_bias(b_ih[1], b_hh[1]))[None, :]
    is4 = np.zeros((4, 32), np.float32)
    for r in range(8):
        for b in range(B_LOC):
            is4[b, 4 * r + b] = 1.0
    is4 = _bf16(is4)
    idb = _bf16(np.eye(128, dtype=np.float32))

    in_maps = []
    for c in range(N_CORES):
        xs = seq_inputs[:t_steps, B_LOC * c : B_LOC * (c + 1), :]  # [T,4,512]
        xt = np.ascontiguousarray(
            xs.transpose(2, 0, 1).reshape(H, t_steps * B_LOC)
        )
        in_maps.append(
            {
                "xt": _bf16(xt),
                "w1i": w1i,
                "w1h": w1h,
                "w2i": w2i,
                "w2h": w2h,
                "b1r": b1,
                "b2r": b2,
                "is4": is4,
                "idb": idb,
            }
        )
    return in_maps


def gather_output(results, t_steps=T_STEPS):
    B = B_LOC * N_CORES
    y = np.empty((t_steps, B, H), np.float32)
    for c in range(N_CORES):
        yc = np.asarray(results[c]["y"], dtype=np.float32)
        yc = yc.reshape(t_steps, 128, 4, 4)  # [t, u_lo, k, b]
        yc = yc.transpose(0, 3, 2, 1).reshape(t_steps, B_LOC, H)
        y[:, B_LOC * c : B_LOC * (c + 1), :] = yc
    return 0.5 * y  # H2 = 2*h2


_NC_CACHE = {}


def kernel(seq_inputs, W_ih, W_hh, b_ih, b_hh):
    seq_inputs = np.asarray(seq_inputs, np.float32)
    W_ih = np.asarray(W_ih, np.float32)
    W_hh = np.asarray(W_hh, np.float32)
    b_ih = np.asarray(b_ih, np.float32)
    b_hh = np.asarray(b_hh, np.float32)
    t_steps = seq_inputs.shape[0]
    if t_steps not in _NC_CACHE:
        _NC_CACHE[t_steps] = build_nc(t_steps)
    nc = _NC_CACHE[t_steps]
    in_maps = host_inputs(seq_inputs, W_ih, W_hh, b_ih, b_hh, t_steps)
    res = run_bass_kernel_spmd(nc, in_maps, core_ids=list(range(N_CORES)))
    return gather_output(res.results, t_steps)


# revision 34
# speedup vs baseline: 1.2092x; 1.0584x over previous
"""Trainium2 Bass kernel for 2-layer LSTM (T=512, B=32, H=512), fp32 I/O.

Strategy: pure data-parallel over batch (8 cores x B_local=4, zero collectives).
Each core runs the full 2-layer scan for its batch slice.

Per-core design:
  - All on-chip layouts are "chunk-partitioned": SBUF/PSUM partition index
    p = 32*k + 4*r + b, where k = hidden-unit chunk (u div 128), r = replica
    (batch replicated 8x to fill the 128-wide PE stationary / all engine
    lanes), b = local batch. Free dim carries (gate, u_lo) for z-shaped
    tiles and u_lo for state-shaped tiles.
  - z matmuls: stationary = transposed hidden state (with replicas), moving =
    pre-transposed weight matrices; 4 PE column-groups stream the 4
    unit-chunks concurrently. Rounds are emitted round-robin across the
    column groups (group-major order would serialize the in-order PE queue).
    A K=5 "inject" round (emitted first: h-independent) adds x-projection
    + bias via a small identity-select stationary.
  - Gates: single ACT Tanh with scale=0.5 over [128, 512] using the
    tanh-half-trick (sigmoid(z) = 0.5*tanh(z/2)+0.5); the doubled-gate
    row scaling is folded into the weights on the host. Cell state is kept
    as S := 2c and hidden as H := 2h so the whole cell update is 4
    scalar_tensor_tensor ops; host folds the compensating 0.5 into all
    H-consuming weight columns and rescales the final output.
  - Each layer's PE transpose of h (the next step's stationary) is emitted
    at the top of that layer's NEXT step, so it never sits mid-queue
    waiting on the DVE gate tail while independent z rounds are blocked
    behind it. The z PSUM tiles are double-buffered so step t's inject can
    stream while step t-1's ACT still reads.
  - x-projections are big GEMMs: layer-1 fully precomputed to DRAM before
    the scan; layer-2 computed in 8-step blocks from transposed H1 tiles
    while the scan runs (layer-2 scan lags layer-1 by LAG=16 steps).
"""

import sys

if "/opt/trn_rl_repo" not in sys.path:
    sys.path.insert(0, "/opt/trn_rl_repo")

import numpy as np
import ml_dtypes

import concourse.bacc as bacc
import concourse.tile as tile
from concourse import mybir
from concourse.bass_utils import run_bass_kernel_spmd

T_STEPS = 512
B_LOC = 4
N_CORES = 8
H = 512
NG = 2048  # 4*H gate width
BLK = 8  # x2proj block size (steps)
LAG = 16  # layer-2 scan lag (steps)
BF = mybir.dt.bfloat16
F32 = mybir.dt.float32

_ADD = mybir.AluOpType.add
_MUL = mybir.AluOpType.mult
_TANH = mybir.ActivationFunctionType.Tanh


def _eff_stream(w, col_scale):
    """w: [2048, d] (gate-major rows i,f,g,o). Returns stream matrix
    [d, 2048] with rows scaled (g-gate x2 for the tanh trick), columns
    reordered chunk-major (n = 512*k + 128*g + u_lo), scaled by col_scale."""
    w2 = w.astype(np.float64) * col_scale
    w2[2 * H : 3 * H] *= 2.0
    wr = w2.reshape(4, 4, 128, w.shape[1])  # [g, k, u_lo, d]
    wr = wr.transpose(1, 0, 2, 3).reshape(NG, w.shape[1])
    return np.ascontiguousarray(wr.T.astype(np.float32))


def _eff_bias(b_ih, b_hh):
    b = (b_ih.astype(np.float64) + b_hh.astype(np.float64)).copy()
    b[2 * H : 3 * H] *= 2.0
    br = b.reshape(4, 4, 128).transpose(1, 0, 2).reshape(NG)
    return br.astype(np.float32)


def _bf16(x):
    return x.astype(ml_dtypes.bfloat16)


def _z_inject(nc, pz, is5, xp_sl):
    """The h-independent xproj inject round (emitted before the epilogue
    transpose so it streams while the PE would otherwise idle)."""
    for j in range(4):
        nc.tensor.matmul(
            pz[32 * j : 32 * j + 32, :],
            is5[:, :],
            xp_sl[0:4, 512 * j : 512 * j + 512],
            start=True,
            stop=False,
            tile_position=(0, 32 * j),
        )


def _z_krounds(nc, pz, ht, w):
    """The 4 recurrent accumulation rounds of z += W.T @ H, round-robin
    across the 4 column-tile groups so their moving streams overlap in
    the PE (group-major order serializes: each group's accumulation
    chain blocks the in-order queue head)."""
    for k in range(4):
        for j in range(4):
            nc.tensor.matmul(
                pz[32 * j : 32 * j + 32, :],
                ht[:, 32 * k : 32 * k + 32],
                w[:, NG * k + 512 * j : NG * k + 512 * j + 512],
                start=False,
                stop=(k == 3),
                tile_position=(0, 32 * j),
            )


def _gates(nc, pools, pz, s_prev, nm):
    """ACT+DVE cell update. Returns (s_new, h_new) tiles."""
    sbuf, state = pools
    t_sb = sbuf.tile([128, 512], BF, tag=f"t{nm}", name=f"t{nm}")
    nc.scalar.activation(t_sb[:], pz[:], _TANH, bias=0.0, scale=0.5)
    m2 = sbuf.tile([128, 128], F32, tag=f"m2{nm}", name=f"m2{nm}")
    nc.vector.scalar_tensor_tensor(
        m2[:], t_sb[:, 0:128], 1.0, t_sb[:, 256:384], op0=_ADD, op1=_MUL
    )
    m1 = sbuf.tile([128, 128], F32, tag=f"m1{nm}", name=f"m1{nm}")
    nc.vector.scalar_tensor_tensor(
        m1[:], t_sb[:, 128:256], 1.0, s_prev[:], op0=_ADD, op1=_MUL
    )
    s_new = state.tile([128, 128], F32, tag=f"s{nm}", name=f"s{nm}")
    nc.vector.scalar_tensor_tensor(
        s_new[:], m1[:], 0.5, m2[:], op0=_MUL, op1=_ADD
    )
    tc_sb = sbuf.tile([128, 128], BF, tag=f"tc{nm}", name=f"tc{nm}")
    nc.scalar.activation(tc_sb[:], s_new[:], _TANH, bias=0.0, scale=0.5)
    h_new = sbuf.tile([128, 128], BF, tag=f"h{nm}", name=f"h{nm}")
    nc.vector.scalar_tensor_tensor(
        h_new[:], t_sb[:, 384:512], 1.0, tc_sb[:], op0=_ADD, op1=_MUL
    )
    return s_new, h_new


def build_nc(t_steps=T_STEPS, repeat=1):
    nc = bacc.Bacc(
        "TRN2", target_bir_lowering=False, debug=False, num_devices=N_CORES
    )
    # kernel inputs (per-core)
    xt_d = nc.dram_tensor("xt", [H, t_steps * B_LOC], BF, kind="ExternalInput")
    w1i_d = nc.dram_tensor("w1i", [H, NG], BF, kind="ExternalInput")
    w1h_d = nc.dram_tensor("w1h", [H, NG], BF, kind="ExternalInput")
    w2i_d = nc.dram_tensor("w2i", [H, NG], BF, kind="ExternalInput")
    w2h_d = nc.dram_tensor("w2h", [H, NG], BF, kind="ExternalInput")
    b1_d = nc.dram_tensor("b1r", [1, NG], BF, kind="ExternalInput")
    b2_d = nc.dram_tensor("b2r", [1, NG], BF, kind="ExternalInput")
    is5_d = nc.dram_tensor("is4", [4, 32], BF, kind="ExternalInput")
    idb_d = nc.dram_tensor("idb", [128, 128], BF, kind="ExternalInput")
    y_d = nc.dram_tensor("y", [t_steps, 2048], BF, kind="ExternalOutput")

    rb_sz = min(128, t_steps * B_LOC)  # phase-1 row-block size
    n_tb = t_steps * B_LOC // rb_sz

    with tile.TileContext(nc) as tc:
        with (
            tc.tile_pool(name="const", bufs=1) as const,
            tc.tile_pool(name="state", bufs=2) as state,
            tc.tile_pool(name="work", bufs=2) as work,
            tc.tile_pool(name="xp1p", bufs=3) as xp1p,
            tc.tile_pool(name="xp2p", bufs=3) as xp2p,
            tc.tile_pool(name="hblkp", bufs=2) as hblkp,
            tc.tile_pool(name="pzp", bufs=2, space="PSUM") as pzp,
            tc.tile_pool(name="ptp", bufs=1, space="PSUM") as ptp,
            tc.tile_pool(name="jnkp", bufs=1, space="PSUM") as jnkp,
            tc.tile_pool(name="dram", bufs=1, space="DRAM") as dramp,
        ):
            # ---- constants / weights to SBUF
            is5 = const.tile([4, 32], BF, name="is4")
            nc.sync.dma_start(is5[:], is5_d.ap())
            b1sb = const.tile([1, NG], BF, name="b1sb")
            nc.sync.dma_start(b1sb[:], b1_d.ap())
            b2sb = const.tile([1, NG], BF, name="b2sb")
            nc.sync.dma_start(b2sb[:], b2_d.ap())
            ones1 = const.tile([1, 128], BF, name="ones1")
            nc.gpsimd.memset(ones1[:], 1.0)
            idb = const.tile([128, 128], BF)
            nc.sync.dma_start(idb[:], idb_d.ap())

            w1h = const.tile([128, 4 * NG], BF, name="w1h")
            w2i = const.tile([128, 4 * NG], BF, name="w2i")
            w2h = const.tile([128, 4 * NG], BF, name="w2h")
            for w_sb, w_dd in ((w1h, w1h_d), (w2i, w2i_d), (w2h, w2h_d)):
                for k in range(4):
                    nc.sync.dma_start(
                        w_sb[:, NG * k : NG * k + NG],
                        w_dd[128 * k : 128 * k + 128, :],
                    )

            # DRAM scratch
            x1d = dramp.tile([t_steps * B_LOC, NG], BF, name="x1d")
            x2d = dramp.tile([t_steps * B_LOC, NG], BF, name="x2d")

            for _rep in range(repeat):
              # ---- phase 1: x1proj GEMM -> DRAM
              with tc.tile_pool(name="ph1", bufs=2) as ph1:
                  w1i = ph1.tile([128, 4 * NG], BF, bufs=1, name="w1i")
                  xts = ph1.tile([128, 4 * t_steps * B_LOC], BF, bufs=1,
                                 name="xts")
                  for k in range(4):
                      nc.sync.dma_start(
                          w1i[:, NG * k : NG * k + NG],
                          w1i_d[128 * k : 128 * k + 128, :],
                      )
                      nc.sync.dma_start(
                          xts[:, t_steps * B_LOC * k : t_steps * B_LOC * (k + 1)],
                          xt_d[128 * k : 128 * k + 128, :],
                      )
                  for tb in range(n_tb):
                      cpx = ph1.tile([rb_sz, NG], BF, name="cpx")
                      for nj in range(4):
                          # borrow the scan's double-buffered z PSUM tags
                          pxa = pzp.tile([rb_sz, 512], F32,
                                         tag=("pz1" if nj % 2 == 0 else "pz2"),
                                         name="pxa")
                          nc.tensor.matmul(
                              pxa[:],
                              ones1[0:1, 0:rb_sz],
                              b1sb[0:1, 512 * nj : 512 * nj + 512],
                              start=True,
                              stop=False,
                          )
                          for k in range(4):
                              nc.tensor.matmul(
                                  pxa[:],
                                  xts[:, t_steps * B_LOC * k + rb_sz * tb :
                                      t_steps * B_LOC * k + rb_sz * tb + rb_sz],
                                  w1i[:, NG * k + 512 * nj : NG * k + 512 * nj + 512],
                                  start=False,
                                  stop=(k == 3),
                              )
                          nc.vector.tensor_copy(
                              cpx[:, 512 * nj : 512 * nj + 512], pxa[:]
                          )
                      nc.sync.dma_start(
                          x1d[rb_sz * tb : rb_sz * tb + rb_sz, :], cpx[:]
                      )

              # ---- initial states
              s1 = state.tile([128, 128], F32, tag="s1", name="s1")
              nc.gpsimd.memset(s1[:], 0.0)
              s2 = state.tile([128, 128], F32, tag="s2", name="s2")
              nc.gpsimd.memset(s2[:], 0.0)
              ht1 = state.tile([128, 128], BF, tag="ht1", name="ht1")
              nc.gpsimd.memset(ht1[:], 0.0)
              ht2 = state.tile([128, 128], BF, tag="ht2", name="ht2")
              nc.gpsimd.memset(ht2[:], 0.0)

              h1c = h2c = hblk = None
              x2_pending = None
              pools = (work, state)

              # HAM warm-keeper: the PE idles ~0.8us before each epilogue
              # transpose (waiting on the DVE gate tail), which re-throttles
              # the PE clock to 1.2GHz. Full-array junk matmuls (K=M=128 so
              # the whole array toggles — small-K/M ops don't register with
              # the activity monitor) in those gaps keep the clock at 2.4GHz;
              # their results are never read.
              junk = jnkp.tile([128, 512], F32, name="junk")

              def _warm(n):
                  for _ in range(n):
                      nc.tensor.matmul(
                          junk[:],
                          idb[:],
                          w1h[:, 0:512],
                          start=True,
                          stop=True,
                      )

              # ---- fused scan (layer epilogues rephased to the next step)
              for tau in range(t_steps + LAG + 1):
                  # --- L1 epilogue for step tau-1: transpose h -> new
                  # stationary, append to the x2proj h-block, fire the
                  # block GEMM when the block completes.
                  t1p = tau - 1
                  if 0 <= t1p < t_steps:
                      pt1 = ptp.tile([128, 128], BF, tag="pt1", name="pt1")
                      nc.tensor.transpose(pt1[:], h1c[:], idb[:])
                      ht1 = state.tile([128, 128], BF, tag="ht1", name="ht1")
                      nc.vector.tensor_copy(ht1[:], pt1[:])
                      bi, off = divmod(t1p, BLK)
                      nc.vector.tensor_copy(
                          hblk[:].rearrange("p (k t b) -> p k t b", k=4, b=4)
                          [:, :, off, :],
                          ht1[:].rearrange("p (k rb) -> p k rb", rb=32)
                          [:, :, 0:4],
                      )
                      if off == BLK - 1:
                          x2_pending = (bi, hblk)

                  # --- L1 step tau
                  if tau < t_steps:
                      t1i = tau
                      if t1i % BLK == 0:
                          hblk = hblkp.tile([128, BLK * 16], BF, tag="hblk",
                                            name="hblk")
                      xp1 = xp1p.tile([B_LOC, NG], BF, tag="xp1", name="xp1")
                      nc.sync.dma_start(
                          xp1[:], x1d[B_LOC * t1i : B_LOC * t1i + B_LOC, :]
                      )
                      pz1 = pzp.tile([128, 512], F32, tag="pz1", name="pz1")
                      _z_inject(nc, pz1, is5, xp1)
                      _z_krounds(nc, pz1, ht1, w1h)
                      s1, h1c = _gates(nc, pools, pz1, s1, "L1")
                      _warm(2)

                  # --- L2 epilogue for step tau-LAG-1: transpose h, emit y
                  t2p = tau - LAG - 1
                  if 0 <= t2p < t_steps:
                      pt2 = ptp.tile([128, 128], BF, tag="pt2", name="pt2")
                      nc.tensor.transpose(pt2[:], h2c[:], idb[:])
                      ht2 = state.tile([128, 128], BF, tag="ht2", name="ht2")
                      nc.vector.tensor_copy(ht2[:], pt2[:])
                      yst = work.tile([128, 16], BF, tag="yst", name="yst")
                      nc.vector.tensor_copy(
                          yst[:].rearrange("u (k b) -> u k b", b=4),
                          ht2[:].rearrange("u (k rb) -> u k rb", rb=32)
                          [:, :, 0:4],
                      )
                      nc.sync.dma_start(
                          y_d[t2p : t2p + 1, :]
                          .rearrange("o (u f) -> (o u) f", u=128),
                          yst[:],
                      )

                  # --- L2 step tau-LAG
                  t2i = tau - LAG
                  if 0 <= t2i < t_steps:
                      xp2 = xp2p.tile([B_LOC, NG], BF, tag="xp2", name="xp2")
                      nc.sync.dma_start(
                          xp2[:], x2d[B_LOC * t2i : B_LOC * t2i + B_LOC, :]
                      )
                      pz2 = pzp.tile([128, 512], F32, tag="pz2", name="pz2")
                      _z_inject(nc, pz2, is5, xp2)
                      _z_krounds(nc, pz2, ht2, w2h)
                      s2, h2c = _gates(nc, pools, pz2, s2, "L2")
                      if x2_pending is None:
                          _warm(2)

                  # --- x2proj GEMM for a completed h-block (emitted last so
                  # it never delays the chain-critical z matmuls; it is
                  # consumed LAG-BLK steps later). 4 col-tiled groups,
                  # round-robin (output chunk nj at PSUM partitions 32*nj).
                  if x2_pending is not None:
                      bi, hb = x2_pending
                      x2_pending = None
                      hbr = hb[:].rearrange("p (k tb) -> p k tb", k=4)
                      cx2 = work.tile([32, NG], BF, tag="cx2", name="cx2")
                      pxb = pzp.tile([128, 512], F32, tag="pxb", bufs=1,
                                     name="pxb")
                      for nj in range(4):
                          nc.tensor.matmul(
                              pxb[32 * nj : 32 * nj + 32, :],
                              ones1[0:1, 0:32],
                              b2sb[0:1, 512 * nj : 512 * nj + 512],
                              start=True,
                              stop=False,
                              tile_position=(0, 32 * nj),
                          )
                      for k in range(4):
                          for nj in range(4):
                              nc.tensor.matmul(
                                  pxb[32 * nj : 32 * nj + 32, :],
                                  hbr[:, k, :],
                                  w2i[:, NG * k + 512 * nj :
                                      NG * k + 512 * nj + 512],
                                  start=False,
                                  stop=(k == 3),
                                  tile_position=(0, 32 * nj),
                              )
                      for nj in range(4):
                          nc.vector.tensor_copy(
                              cx2[:, 512 * nj : 512 * nj + 512],
                              pxb[32 * nj : 32 * nj + 32, :],
                          )
                      nc.sync.dma_start(
                          x2d[B_LOC * BLK * bi : B_LOC * BLK * (bi + 1), :],
                          cx2[:],
                      )
    nc.compile()
    return nc


def host_inputs(seq_inputs, W_ih, W_hh, b_ih, b_hh, t_steps=T_STEPS):
    """Build the 8 per-core input maps."""
    w1i = _bf16(_eff_stream(W_ih[0], 1.0))
    w1h = _bf16(_eff_stream(W_hh[0], 0.5))
    w2i = _bf16(_eff_stream(W_ih[1], 0.5))
    w2h = _bf16(_eff_stream(W_hh[1], 0.5))
    b1 = _bf16(_eff_bias(b_ih[0], b_hh[0]))[None, :]
    b2 = _bf16(_eff_bias(b_ih[1], b_hh[1]))[None, :]
    is4 = np.zeros((4, 32), np.float32)
    for r in range(8):
        for b in range(B_LOC):
            is4[b, 4 * r + b] = 1.0
    is4 = _bf16(is4)
    idb = _bf16(np.eye(128, dtype=np.float32))

    in_maps = []
    for c in range(N_CORES):
        xs = seq_inputs[:t_steps, B_LOC * c : B_LOC * (c + 1), :]  # [T,4,512]
        xt = np.ascontiguousarray(
            xs.transpose(2, 0, 1).reshape(H, t_steps * B_LOC)
        )
        in_maps.append(
            {
                "xt": _bf16(xt),
                "w1i": w1i,
                "w1h": w1h,
                "w2i": w2i,
                "w2h": w2h,
                "b1r": b1,
                "b2r": b2,
                "is4": is4,
                "idb": idb,
            }
        )
    return in_maps


def gather_output(results, t_steps=T_STEPS):
    B = B_LOC * N_CORES
    y = np.empty((t_steps, B, H), np.float32)
    for c in range(N_CORES):
        yc = np.asarray(results[c]["y"], dtype=np.float32)
        yc = yc.reshape(t_steps, 128, 4, 4)  # [t, u_lo, k, b]
        yc = yc.transpose(0, 3, 2, 1).reshape(t_steps, B_LOC, H)
        y[:, B_LOC * c : B_LOC * (c + 1), :] = yc
    return 0.5 * y  # H2 = 2*h2


_NC_CACHE = {}


def kernel(seq_inputs, W_ih, W_hh, b_ih, b_hh):
    seq_inputs = np.asarray(seq_inputs, np.float32)
    W_ih = np.asarray(W_ih, np.float32)
    W_hh = np.asarray(W_hh, np.float32)
    b_ih = np.asarray(b_ih, np.float32)
    b_hh = np.asarray(b_hh, np.float32)
    t_steps = seq_inputs.shape[0]
    if t_steps not in _NC_CACHE:
        _NC_CACHE[t_steps] = build_nc(t_steps)
    nc = _NC_CACHE[t_steps]
    in_maps = host_inputs(seq_inputs, W_ih, W_hh, b_ih, b_hh, t_steps)
    res = run_bass_kernel_spmd(nc, in_maps, core_ids=list(range(N_CORES)))
    return gather_output(res.results, t_steps)
